# revision 4
# baseline (speedup 1.0000x reference)
"""Trainium2 Bass kernel for nn_HandwritingLNNAttention — sparse-synapse version.

LTC RNN, 96-step scan x 6 ODE unfolds. Data parallel: batch 1024 -> 128/core
across 8 cores.

Per-core layout: everything in [unit, batch] ([u,b]) space, v state fp16.
The recurrent synapses are ~30% dense (mask); active (i,u) pairs are packed
at compile time into K slots sorted by u, J = ceil(K/128) chunks of 128:
  - gather+scale: psum_args[k,b] = sum_i P_j[i,k] * v_T[i,b], P one-hot rows
    scaled by sigma[i_k,u_k]  (PE, fp16)
  - args = psum_args + msig (DVE, writes fp16), sigmoid on ACT (fp16)
  - num/den reductions: PSUM accumulation chains over chunks with one-hot
    column matrices carrying wpe/wp (PE)
  - cm_t*v, and the per-step sensory/leak totals, are injected into the same
    PSUM accumulation via diag/identity matmuls
  - update: v_T = num * reciprocal(den)  (DVE, fp16 out)
Sensory path + LN + attention/classifier epilogue mirror the dense baseline.
"""

import sys
import numpy as np

try:
    import concourse.bass as bass
except ImportError:  # pragma: no cover
    sys.path.insert(0, "/opt/trn_rl_repo")
    import concourse.bass as bass

import concourse.tile as tile
from concourse import bacc, bass_utils, mybir

F32 = mybir.dt.float32
F16 = mybir.dt.float16
AF = mybir.ActivationFunctionType
OP = mybir.AluOpType

N_CORES = 8
B_FULL = 1024
B = B_FULL // N_CORES  # 128 per core
T = 96
I = 6
U = 128
M = 64   # motor units
H1 = 32  # attention hidden
H2 = 128  # classifier hidden
C = 100
UNFOLDS = 6
EPS = 1e-8

TRACE = False
LAST_RESULTS = None


def _softplus(x):
    return np.log1p(np.exp(-np.abs(x))) + np.maximum(x, 0.0)


def _build_params(inputs):
    """Numpy-side parameter preprocessing (fp64 -> fp32/fp16)."""
    f = lambda a: np.ascontiguousarray(a, dtype=np.float32)
    h = lambda a: np.ascontiguousarray(a, dtype=np.float16)
    gleak = np.asarray(inputs["gleak"], np.float64)
    vleak = np.asarray(inputs["vleak"], np.float64)
    cm = np.asarray(inputs["cm"], np.float64)
    sigma = np.asarray(inputs["sigma"], np.float64)
    mu = np.asarray(inputs["mu"], np.float64)
    w = np.asarray(inputs["w"], np.float64)
    erev = np.asarray(inputs["erev"], np.float64)
    mask = np.asarray(inputs["mask"], np.float64)
    s_sigma = np.asarray(inputs["sens_sigma"], np.float64)
    s_mu = np.asarray(inputs["sens_mu"], np.float64)
    s_w = np.asarray(inputs["sens_w"], np.float64)
    s_erev = np.asarray(inputs["sens_erev"], np.float64)
    s_mask = np.asarray(inputs["sens_mask"], np.float64)

    cm_t = _softplus(cm) * UNFOLDS          # [U]
    gl = _softplus(gleak)                   # [U]
    wp = _softplus(w) * mask                # [U,U] (i,u)
    wpe = wp * erev
    swp = _softplus(s_w) * s_mask           # [I,U]
    swpe = swp * s_erev
    msig_d = -(mu * sigma)                  # [U,U]

    # ---- sparse packing of active recurrent synapses, sorted by (u, i) ----
    uu, ii = np.nonzero(mask.T)             # sorted by u then i
    K = len(uu)
    J = max(1, (K + 127) // 128)
    Kp = J * 128
    P_np = np.zeros((U, J, 128), np.float16)
    Qn_np = np.zeros((128, J, U), np.float16)
    Qd_np = np.zeros((128, J, U), np.float16)
    msig_np = np.zeros((128, J), np.float32)
    for k in range(K):
        j, c = divmod(k, 128)
        i_k, u_k = int(ii[k]), int(uu[k])
        P_np[i_k, j, c] = sigma[i_k, u_k]
        Qn_np[c, j, u_k] = wpe[i_k, u_k]
        Qd_np[c, j, u_k] = wp[i_k, u_k]
        msig_np[c, j] = msig_d[i_k, u_k]

    p = {}
    p["_J"] = J  # not a tensor; popped before upload
    p["P"] = P_np
    p["Qn"] = Qn_np
    p["Qd"] = Qd_np
    p["msig"] = msig_np
    p["msig_rows"] = h(msig_np.T.reshape(1, J, 128))  # [1, J, 128] for rank-1 bias mms
    p["ones_row"] = h(np.ones((1, 128)))
    p["diagcmt"] = h(np.diag(cm_t))
    p["ident16"] = h(np.eye(128))
    p["glvl_b"] = f(np.broadcast_to((gl * vleak)[None, :], (B, U)))
    p["cmgl_eps_b"] = f(np.broadcast_to((cm_t + gl + EPS)[None, :], (B, U)))
    # sensory slabs in [b, u, i'] layout, broadcast over b
    p["ssig_s"] = f(np.broadcast_to(s_sigma.T[None], (B, U, I)))
    p["smsig_s"] = f(np.broadcast_to((-(s_mu * s_sigma)).T[None], (B, U, I)))
    p["swpe_s"] = f(np.broadcast_to(swpe.T[None], (B, U, I)))
    p["swp_s"] = f(np.broadcast_to(swp.T[None], (B, U, I)))
    # layernorm / input affine folded: inp = xn*effg + effb per feature
    effg = np.asarray(inputs["ln_g"], np.float64) * np.asarray(inputs["in_w"], np.float64)
    effb = (np.asarray(inputs["ln_b"], np.float64) * np.asarray(inputs["in_w"], np.float64)
            + np.asarray(inputs["in_b"], np.float64))
    p["effg_rep"] = f(np.broadcast_to(effg[None, None, :], (B, T, I)))
    p["effb_rep"] = f(np.broadcast_to(effb[None, None, :], (B, T, I)))
    p["outw"] = f(np.asarray(inputs["out_w"]).reshape(M, 1))
    p["outb"] = f(np.asarray(inputs["out_b"]).reshape(M, 1))
    p["aw1"] = f(inputs["aw1"])                       # [64,32]
    p["ab1"] = f(np.asarray(inputs["ab1"]).reshape(H1, 1))
    p["aw2"] = f(inputs["aw2"])                       # [32,1]
    p["cw1"] = f(inputs["cw1"])                       # [64,128]
    p["cb1"] = f(np.asarray(inputs["cb1"]).reshape(H2, 1))
    p["cw2"] = f(inputs["cw2"])                       # [128,100]
    p["cb2"] = f(np.asarray(inputs["cb2"]).reshape(C, 1))
    p["ident"] = f(np.eye(128))
    p["ones_m"] = f(np.ones((1, M)))
    return p


def _declare_inputs(nc, p):
    d = {}
    for name, arr in p.items():
        if name.startswith("_"):
            continue
        dt = F16 if arr.dtype == np.float16 else F32
        d[name] = nc.dram_tensor(name, list(arr.shape), dt, kind="ExternalInput").ap()
    d["x"] = nc.dram_tensor("x", [B, T, I], F32, kind="ExternalInput").ap()
    return d


def _build(nc, tc, d, J, n_steps=T, hw_loop=True, dbg=False):
    out_d = nc.dram_tensor("out", [B, C], F32, kind="ExternalOutput").ap()
    if dbg:
        dbg_v = nc.dram_tensor("dbg_v", [U, B], F16, kind="ExternalOutput").ap()
        dbg_wnT = nc.dram_tensor("dbg_wnT", [U, B], F16, kind="ExternalOutput").ap()
        dbg_wdT = nc.dram_tensor("dbg_wdT", [U, B], F16, kind="ExternalOutput").ap()
        dbg_outs = nc.dram_tensor("dbg_outs", [M, B, T], F32, kind="ExternalOutput").ap()
        dbg_inp = nc.dram_tensor("dbg_inp", [B, T + 1, I], F32, kind="ExternalOutput").ap()

    # chunking of the J k-chunks for args/sigmoid processing: small chunks at
    # the unfold boundaries shorten the serial gather->add->sigmoid->reduce
    # chain that restarts each unfold; 6-wide chunks in the middle.
    CH = 6
    AHEAD = 3  # gather-ahead depth (PSUM-buffer limited)
    if J > 12:
        rem = J - 5
        m, r = divmod(rem, 6)
        sizes = [3] + [6] * m + ([r] if r else []) + [2]
    else:
        sizes = [min(CH, J - j0) for j0 in range(0, J, CH)]
    chunks = []
    j0 = 0
    for n in sizes:
        chunks.append((j0, n))
        j0 += n
    assert j0 == J, (sizes, J)

    cpool = tc.alloc_tile_pool(name="consts", bufs=1)
    P_sb = cpool.tile([U, J, 128], F16)
    Qn_sb = cpool.tile([128, J, U], F16)
    Qd_sb = cpool.tile([128, J, U], F16)
    msig_sb = cpool.tile([128, J], F32)
    msig_rows_sb = cpool.tile([1, J, 128], F16)
    ones_row_sb = cpool.tile([1, 128], F16)
    diagcmt_sb = cpool.tile([128, 128], F16)
    ident16_sb = cpool.tile([128, 128], F16)
    glvl_b = cpool.tile([B, U], F32)
    cmgl_eps_b = cpool.tile([B, U], F32)
    ssig_s = cpool.tile([B, U, I], F32)
    smsig_s = cpool.tile([B, U, I], F32)
    swpe_s = cpool.tile([B, U, I], F32)
    swp_s = cpool.tile([B, U, I], F32)
    outw_sb = cpool.tile([M, 1], F32)
    outb_sb = cpool.tile([M, 1], F32)
    ident_sb = cpool.tile([128, 128], F32)
    for t_sb, name in [(P_sb, "P"), (Qn_sb, "Qn"), (Qd_sb, "Qd"), (msig_sb, "msig"),
                       (msig_rows_sb, "msig_rows"), (ones_row_sb, "ones_row"),
                       (diagcmt_sb, "diagcmt"), (ident16_sb, "ident16"),
                       (glvl_b, "glvl_b"), (cmgl_eps_b, "cmgl_eps_b"),
                       (ssig_s, "ssig_s"), (smsig_s, "smsig_s"), (swpe_s, "swpe_s"),
                       (swp_s, "swp_s"), (outw_sb, "outw"), (outb_sb, "outb"),
                       (ident_sb, "ident")]:
        nc.sync.dma_start(out=t_sb[:], in_=d[name])

    # ---------------- LN prologue -> inp slab [B, T+1, I] (last step zero) ----
    inp_slab = cpool.tile([B, T + 1, I], F32)
    nc.vector.memset(inp_slab[:], 0.0)
    x_sb = cpool.tile([B, T, I], F32)
    nc.sync.dma_start(out=x_sb[:], in_=d["x"])
    effg_sb = cpool.tile([B, T, I], F32)
    effb_sb = cpool.tile([B, T, I], F32)
    nc.sync.dma_start(out=effg_sb[:], in_=d["effg_rep"])
    nc.sync.dma_start(out=effb_sb[:], in_=d["effb_rep"])

    lnp = tc.alloc_tile_pool(name="ln", bufs=1)
    mean = lnp.tile([B, T, 1], F32)
    nc.vector.reduce_sum(mean[:, :, 0], x_sb[:], mybir.AxisListType.X)
    nc.vector.tensor_scalar_mul(mean[:], mean[:], 1.0 / I)
    xc = lnp.tile([B, T, I], F32)
    nc.vector.tensor_sub(xc[:], x_sb[:], mean[:].to_broadcast((B, T, I)))
    sq = lnp.tile([B, T, I], F32)
    nc.vector.tensor_mul(sq[:], xc[:], xc[:])
    ms = lnp.tile([B, T, 1], F32)
    nc.vector.reduce_sum(ms[:, :, 0], sq[:], mybir.AxisListType.X)
    sd = lnp.tile([B, T, 1], F32)
    ln_eps = lnp.tile([B, 1], F32)
    nc.vector.memset(ln_eps[:], 1e-5)
    nc.scalar.activation(sd[:], ms[:], AF.Sqrt, bias=ln_eps[:], scale=1.0 / I)
    rstd = lnp.tile([B, T, 1], F32)
    nc.vector.reciprocal(rstd[:], sd[:])
    xn = lnp.tile([B, T, I], F32)
    nc.vector.tensor_mul(xn[:], xc[:], rstd[:].to_broadcast((B, T, I)))
    nc.vector.tensor_mul(xn[:], xn[:], effg_sb[:])
    nc.vector.tensor_add(inp_slab[:, 0:T, :], xn[:], effb_sb[:])
    lnp.release()

    # ---------------- scan state ----------------
    v_T = cpool.tile([U, B], F16)
    nc.vector.memset(v_T[:], 0.0)
    outs_T = cpool.tile([M, B, T], F32)
    if dbg:
        nc.vector.memset(outs_T[:], 0.0)
    wnum_tot = cpool.tile([B, U], F32)   # [b,u] sens+leak totals
    wden_tot = cpool.tile([B, U], F32)
    wnumT = cpool.tile([U, B], F16)      # transposed for PE injection
    wdenT = cpool.tile([U, B], F16)

    spool = tc.alloc_tile_pool(name="sens", bufs=2)
    apool = tc.alloc_tile_pool(name="args", bufs=3)
    gpool = tc.alloc_tile_pool(name="g", bufs=3)
    pa_pool = tc.alloc_tile_pool(name="pa", bufs=AHEAD, space="PSUM")
    nd_pool = tc.alloc_tile_pool(name="pnd", bufs=2, space="PSUM")

    def sens_stages(t_idx):
        """Stage list computing wnum_tot/wden_tot ([b,u]) for step t_idx.

        Split into small pieces interleaved between unfolds so the big Pool
        ops never head-of-line block the unfold-chunk adds on the Pool queue.
        """
        inp_t = inp_slab[:, bass.ds(t_idx, 1), :]  # [B, 1, I] -> broadcast over u
        sarg = spool.tile([B, U, I], F32, name=f"sarg{t_idx if isinstance(t_idx, int) else 'r'}")
        ssg = spool.tile([B, U, I], F32, name="ssg_t")
        tmp = spool.tile([B, U, I], F32, name="tmp_t")
        tmp2 = spool.tile([B, U, I], F32, name="tmp2_t")
        wns = spool.tile([B, U, 1], F32, name="wns_t")
        wds = spool.tile([B, U, 1], F32, name="wds_t")
        return [
            lambda: nc.gpsimd.tensor_mul(sarg[:], inp_t.to_broadcast((B, U, I)), ssig_s[:]),
            lambda: (nc.gpsimd.tensor_add(sarg[:], sarg[:], smsig_s[:]),
                     nc.scalar.activation(ssg[:], sarg[:], AF.Sigmoid)),
            lambda: (nc.gpsimd.tensor_mul(tmp[:], ssg[:], swpe_s[:]),
                     nc.vector.reduce_sum(wns[:, :, 0], tmp[:], mybir.AxisListType.X)),
            lambda: (nc.gpsimd.tensor_mul(tmp2[:], ssg[:], swp_s[:]),
                     nc.vector.reduce_sum(wds[:, :, 0], tmp2[:], mybir.AxisListType.X)),
            lambda: (nc.gpsimd.tensor_add(wnum_tot[:], wns[:, :, 0], glvl_b[:]),
                     nc.gpsimd.tensor_add(wden_tot[:], wds[:, :, 0], cmgl_eps_b[:])),
        ]

    def sens_block(t_idx):
        for f in sens_stages(t_idx):
            f()

    def sens_transpose():
        # transpose to [u, b] fp16 for PE injection
        pt1 = nd_pool.tile([U, B], F32, tag="ps")
        nc.tensor.transpose(pt1[:], wnum_tot[:], ident_sb[:])
        nc.vector.tensor_copy(wnumT[:], pt1[:])
        pt2 = nd_pool.tile([U, B], F32, tag="ps")
        nc.tensor.transpose(pt2[:], wden_tot[:], ident_sb[:])
        nc.vector.tensor_copy(wdenT[:], pt2[:])

    sens_block(0)
    sens_transpose()

    PEBIAS = 3  # leading chunks whose msig lands via PE rank-1 matmuls

    def gathers(pa, j0, n, with_bias=False):
        for jl in range(n):
            j = j0 + jl
            if with_bias:
                # args = P_j^T v + msig_j assembled fully in PSUM: ACT can
                # read it directly, keeping DVE off the unfold-restart chain
                nc.tensor.matmul(pa[:, jl, :], lhsT=P_sb[:, j, :], rhs=v_T[:],
                                 start=True, stop=False)
                nc.tensor.matmul(pa[:, jl, :], lhsT=msig_rows_sb[:, j, :],
                                 rhs=ones_row_sb[:], start=False, stop=True)
            else:
                nc.tensor.matmul(pa[:, jl, :], lhsT=P_sb[:, j, :], rhs=v_T[:],
                                 start=True, stop=True)

    def unfold_body(transpose_mid=False):
        nd = nd_pool.tile([U, 2, B], F32, tag="ps")  # [:,0,:]=num, [:,1,:]=den
        pas = {}
        # first gathers go ahead of the injects on the PE queue: the sigmoid
        # pipeline restart is the critical chain after the v update
        pas[0] = pa_pool.tile([128, CH, B], F32, tag="ps", name="pa0")
        gathers(pas[0], chunks[0][0], chunks[0][1], with_bias=0 < PEBIAS)
        # num+den share one PSUM bank => a single accumulation group: the
        # bank-wide start from the diag matmul covers den (its first write
        # lands on pending-zero bytes), and only the last Qd carries stop.
        nc.tensor.matmul(nd[:, 0, :], lhsT=diagcmt_sb[:], rhs=v_T[:],
                         start=True, stop=False)
        nc.tensor.matmul(nd[:, 0, :], lhsT=ident16_sb[:], rhs=wnumT[:],
                         start=False, stop=False)
        nc.tensor.matmul(nd[:, 1, :], lhsT=ident16_sb[:], rhs=wdenT[:],
                         start=False, stop=False)
        for ci in range(1, min(AHEAD, len(chunks))):
            pas[ci] = pa_pool.tile([128, CH, B], F32, tag="ps", name=f"pa{ci}")
            gathers(pas[ci], chunks[ci][0], chunks[ci][1], with_bias=ci < PEBIAS)
        def msig_bc(j0, n0, n1):
            return (msig_sb[:, j0 + n0:j0 + n1]
                    .rearrange("p (j o) -> p j o", o=1).to_broadcast((128, n1 - n0, B)))
        for ci, (j0, n) in enumerate(chunks):
            pa = pas.pop(ci)
            if ci < PEBIAS:
                g_src = pa[:, 0:n, :]  # bias already injected on PE
            else:
                # GPSIMD cannot access PSUM (neuronxcc BIR verifier) -> DVE
                args_sb = apool.tile([128, CH, B], F16)
                nc.vector.tensor_add(args_sb[:, 0:n, :], pa[:, 0:n, :], msig_bc(j0, 0, n))
                g_src = args_sb[:, 0:n, :]
            g_sb = gpool.tile([128, CH, B], F16)
            nc.scalar.activation(g_sb[:, 0:n, :], g_src, AF.Sigmoid)
            if transpose_mid and ci == 1:
                # next step's sens transposes: after this unfold's injects
                # (last wnumT readers), away from the fill/drain chains
                sens_transpose()
            if ci + AHEAD < len(chunks):
                pas[ci + AHEAD] = pa_pool.tile([128, CH, B], F32, tag="ps", name=f"pa{ci + AHEAD}")
                gathers(pas[ci + AHEAD], chunks[ci + AHEAD][0], chunks[ci + AHEAD][1],
                        with_bias=ci + AHEAD < PEBIAS)
            for jl in range(n):
                j = j0 + jl
                last = j == J - 1
                nc.tensor.matmul(nd[:, 0, :], lhsT=Qn_sb[:, j, :], rhs=g_sb[:, jl, :],
                                 start=False, stop=False)
                nc.tensor.matmul(nd[:, 1, :], lhsT=Qd_sb[:, j, :], rhs=g_sb[:, jl, :],
                                 start=False, stop=last)
        # v = num/den: two instructions — DVE may read only one PSUM operand
        # per instruction on HW (NCC_IBVF027)
        rec = apool.tile([U, B], F32, name="rec")
        nc.vector.reciprocal(rec[:], nd[:, 1, :])
        nc.vector.tensor_mul(v_T[:], nd[:, 0, :], rec[:])

    def step_body(t):
        for _k in range(UNFOLDS - 1):
            unfold_body()
        # sensory precompute for t+1 overlaps the unfolds; its transposes are
        # emitted inside the last unfold, right after the final wnumT readers
        sens_block(t + 1)
        unfold_body(transpose_mid=True)
        # outs_T[:, :, t] = v_T[:64] * out_w + out_b
        nc.vector.tensor_scalar(
            out=outs_T[:, :, bass.ds(t, 1)],
            in0=v_T[0:M, :].rearrange("p (b o) -> p b o", o=1),
            scalar1=outw_sb[:], scalar2=outb_sb[:], op0=OP.mult, op1=OP.add)

    if hw_loop:
        with tc.For_i(0, n_steps, 1) as t:
            step_body(t)
    else:
        for t in range(n_steps):
            step_body(t)

    if dbg:
        nc.sync.dma_start(out=dbg_v, in_=v_T[:])
        nc.sync.dma_start(out=dbg_wnT, in_=wnumT[:])
        nc.sync.dma_start(out=dbg_wdT, in_=wdenT[:])
        nc.sync.dma_start(out=dbg_outs, in_=outs_T[:])
        nc.sync.dma_start(out=dbg_inp, in_=inp_slab[:])

    for pool in (nd_pool, pa_pool, gpool, apool, spool):
        pool.release()

    # ---------------- attention pooling + classifier ----------------
    aw1_sb = cpool.tile([M, H1], F32)
    ab1_sb = cpool.tile([H1, 1], F32)
    aw2_sb = cpool.tile([H1, 1], F32)
    cw1_sb = cpool.tile([M, H2], F32)
    cb1_sb = cpool.tile([H2, 1], F32)
    cw2_sb = cpool.tile([H2, C], F32)
    cb2_sb = cpool.tile([C, 1], F32)
    ones_sb = cpool.tile([1, M], F32)
    for t_sb, name in [(aw1_sb, "aw1"), (ab1_sb, "ab1"), (aw2_sb, "aw2"),
                       (cw1_sb, "cw1"), (cb1_sb, "cb1"), (cw2_sb, "cw2"),
                       (cb2_sb, "cb2"), (ones_sb, "ones_m")]:
        nc.sync.dma_start(out=t_sb[:], in_=d[name])

    epool = tc.alloc_tile_pool(name="ep", bufs=2)
    e1pool = tc.alloc_tile_pool(name="e1", bufs=1)
    ps_h = tc.alloc_tile_pool(name="psh", bufs=2, space="PSUM")
    ps_s = tc.alloc_tile_pool(name="pss", bufs=2, space="PSUM")

    outs_flat = outs_T[:].rearrange("p b t -> p (b t)")
    dpool = tc.alloc_tile_pool(name="dscr", bufs=1, space="DRAM")
    scr1 = dpool.tile([1, B * T], F32)
    NC1 = 512
    for c in range(B * T // NC1):
        hp = ps_h.tile([H1, NC1], F32, tag="ps")
        nc.tensor.matmul(hp[:], lhsT=aw1_sb[:], rhs=outs_flat[:, c * NC1:(c + 1) * NC1],
                         start=True, stop=True)
        hs = epool.tile([H1, NC1], F32)
        nc.scalar.activation(hs[:], hp[:], AF.Relu, bias=ab1_sb[:])
        sp = ps_s.tile([1, NC1], F32, tag="ps")
        nc.tensor.matmul(sp[:], lhsT=aw2_sb[:], rhs=hs[:], start=True, stop=True)
        sc = epool.tile([1, NC1], F32)
        nc.vector.tensor_copy(sc[:], sp[:])
        nc.sync.dma_start(out=scr1[:, c * NC1:(c + 1) * NC1], in_=sc[:])

    # softmax over t, per b: scores land in DRAM as [1, (b t)]; reload as [b, t]
    scores_bt = e1pool.tile([B, T], F32)
    nc.sync.dma_start(out=scores_bt[:],
                      in_=scr1[:].rearrange("o (b t) -> (o b) t", b=B))
    mx = e1pool.tile([B, 1], F32)
    nc.vector.reduce_max(mx[:], scores_bt[:], mybir.AxisListType.X)
    es = e1pool.tile([B, T], F32)
    nc.vector.tensor_scalar(out=es[:], in0=scores_bt[:], scalar1=mx[:],
                            scalar2=None, op0=OP.subtract)
    nc.scalar.activation(es[:], es[:], AF.Exp)
    ssum = e1pool.tile([B, 1], F32)
    nc.vector.reduce_sum(ssum[:], es[:], mybir.AxisListType.X)
    rs = e1pool.tile([B, 1], F32)
    nc.vector.reciprocal(rs[:], ssum[:])
    attn_bt = e1pool.tile([B, T], F32)
    nc.vector.tensor_scalar(out=attn_bt[:], in0=es[:], scalar1=rs[:],
                            scalar2=None, op0=OP.mult)
    scr2 = dpool.tile([B, T], F32)
    nc.sync.dma_start(out=scr2[:], in_=attn_bt[:])
    attn_flat = e1pool.tile([1, B * T], F32)
    nc.sync.dma_start(out=attn_flat[:], in_=scr2[:].rearrange("b t -> (b t)").rearrange("(o n) -> o n", o=1))

    # ctx_T[m, b] = sum_t outs_T[m,b,t] * attn[b,t]
    ctx_T = e1pool.tile([M, B], F32)
    NB = 4
    for c in range(B // NB):
        ap_ps = ps_h.tile([M, NB * T], F32, tag="ps")
        nc.tensor.matmul(ap_ps[:], lhsT=ones_sb[:],
                         rhs=attn_flat[:, c * NB * T:(c + 1) * NB * T],
                         start=True, stop=True)
        wo = epool.tile([M, NB, T], F32)
        nc.vector.tensor_mul(wo[:], outs_T[:, c * NB:(c + 1) * NB, :],
                             ap_ps[:].rearrange("p (b t) -> p b t", t=T))
        nc.vector.reduce_sum(ctx_T[:, c * NB:(c + 1) * NB], wo[:], mybir.AxisListType.X)

    # classifier
    h2p = ps_h.tile([H2, B], F32, tag="ps")
    nc.tensor.matmul(h2p[:], lhsT=cw1_sb[:], rhs=ctx_T[:], start=True, stop=True)
    h2 = e1pool.tile([H2, B], F32)
    nc.scalar.activation(h2[:], h2p[:], AF.Relu, bias=cb1_sb[:])
    zp = ps_h.tile([C, B], F32, tag="ps")
    nc.tensor.matmul(zp[:], lhsT=cw2_sb[:], rhs=h2[:], start=True, stop=True)
    zT = e1pool.tile([C, B], F32)
    nc.scalar.activation(zT[:], zp[:], AF.Identity, bias=cb2_sb[:])
    tp = ps_h.tile([B, C], F32, tag="ps")
    nc.tensor.matmul(tp[:], lhsT=zT[:], rhs=ident_sb[0:C, 0:C], is_transpose=True,
                     start=True, stop=True)
    zf = e1pool.tile([B, C], F32)
    nc.vector.tensor_copy(zf[:], tp[:])
    nc.sync.dma_start(out=out_d, in_=zf[:])

    for pool in (dpool, ps_s, ps_h, e1pool, epool, cpool):
        pool.release()


_CACHE = {}


def _get_compiled(p, n_steps=T, hw_loop=True, dbg=False):
    key = ("nc", n_steps, hw_loop, dbg)
    if key in _CACHE:
        return _CACHE[key]
    nc = bacc.Bacc("TRN2", target_bir_lowering=False, debug=False,
                   enable_asserts=False)
    d = _declare_inputs(nc, p)
    with tile.TileContext(nc) as tc:
        _build(nc, tc, d, p["_J"], n_steps=n_steps, hw_loop=hw_loop, dbg=dbg)
    nc.compile()
    _CACHE[key] = nc
    return nc


def kernel(**inputs):
    global LAST_RESULTS
    p = _build_params(inputs)
    nc = _get_compiled(p)
    x = np.ascontiguousarray(np.asarray(inputs["x"], np.float32))
    pt = {k: v for k, v in p.items() if not k.startswith("_")}
    in_maps = []
    for ci in range(N_CORES):
        m = dict(pt)
        m["x"] = np.ascontiguousarray(x[ci * B:(ci + 1) * B])
        in_maps.append(m)
    res = bass_utils.run_bass_kernel_spmd(
        nc, in_maps, core_ids=list(range(N_CORES)), trace=TRACE)
    LAST_RESULTS = res
    out = np.concatenate([res.results[ci]["out"] for ci in range(N_CORES)], axis=0)
    return out.astype(np.float32)


# revision 7
# speedup vs baseline: 1.1022x; 1.1022x over previous
"""Trainium2 Bass kernel for nn_HandwritingLNNAttention — sparse-synapse version.

LTC RNN, 96-step scan x 6 ODE unfolds. Data parallel: batch 1024 -> 128/core
across 8 cores.

Per-core layout: everything in [unit, batch] ([u,b]) space, v state fp16.
The recurrent synapses are ~30% dense (mask); active (i,u) pairs are packed
at compile time into K slots sorted by u, J = ceil(K/128) chunks of 128:
  - gather+scale: psum_args[k,b] = sum_i P_j[i,k] * v_T[i,b], P one-hot rows
    scaled by sigma[i_k,u_k]  (PE, fp16)
  - args = psum_args + msig (DVE, writes fp16), sigmoid on ACT (fp16)
  - num/den reductions: PSUM accumulation chains over chunks with one-hot
    column matrices carrying wpe/wp (PE)
  - cm_t*v, and the per-step sensory/leak totals, are injected into the same
    PSUM accumulation via diag/identity matmuls
  - update: v_T = num * reciprocal(den)  (DVE, fp16 out)
Sensory path + LN + attention/classifier epilogue mirror the dense baseline.
"""

import sys
import numpy as np

try:
    import concourse.bass as bass
except ImportError:  # pragma: no cover
    sys.path.insert(0, "/opt/trn_rl_repo")
    import concourse.bass as bass

import concourse.tile as tile
from concourse import bacc, bass_utils, mybir

F32 = mybir.dt.float32
F16 = mybir.dt.float16
AF = mybir.ActivationFunctionType
OP = mybir.AluOpType

N_CORES = 8
B_FULL = 1024
B = B_FULL // N_CORES  # 128 per core
T = 96
I = 6
U = 128
M = 64   # motor units
H1 = 32  # attention hidden
H2 = 128  # classifier hidden
C = 100
UNFOLDS = 6
EPS = 1e-8

TRACE = False
LAST_RESULTS = None


def _softplus(x):
    return np.log1p(np.exp(-np.abs(x))) + np.maximum(x, 0.0)


def _build_params(inputs):
    """Numpy-side parameter preprocessing (fp64 -> fp32/fp16)."""
    f = lambda a: np.ascontiguousarray(a, dtype=np.float32)
    h = lambda a: np.ascontiguousarray(a, dtype=np.float16)
    gleak = np.asarray(inputs["gleak"], np.float64)
    vleak = np.asarray(inputs["vleak"], np.float64)
    cm = np.asarray(inputs["cm"], np.float64)
    sigma = np.asarray(inputs["sigma"], np.float64)
    mu = np.asarray(inputs["mu"], np.float64)
    w = np.asarray(inputs["w"], np.float64)
    erev = np.asarray(inputs["erev"], np.float64)
    mask = np.asarray(inputs["mask"], np.float64)
    s_sigma = np.asarray(inputs["sens_sigma"], np.float64)
    s_mu = np.asarray(inputs["sens_mu"], np.float64)
    s_w = np.asarray(inputs["sens_w"], np.float64)
    s_erev = np.asarray(inputs["sens_erev"], np.float64)
    s_mask = np.asarray(inputs["sens_mask"], np.float64)

    cm_t = _softplus(cm) * UNFOLDS          # [U]
    gl = _softplus(gleak)                   # [U]
    wp = _softplus(w) * mask                # [U,U] (i,u)
    wpe = wp * erev
    swp = _softplus(s_w) * s_mask           # [I,U]
    swpe = swp * s_erev
    msig_d = -(mu * sigma)                  # [U,U]

    # ---- sparse packing of active recurrent synapses, sorted by (u, i) ----
    uu, ii = np.nonzero(mask.T)             # sorted by u then i
    K = len(uu)
    J = max(1, (K + 127) // 128)
    Kp = J * 128
    P_np = np.zeros((U, J, 128), np.float16)
    Qn_np = np.zeros((128, J, U), np.float16)
    Qd_np = np.zeros((128, J, U), np.float16)
    msig_np = np.zeros((128, J), np.float32)
    for k in range(K):
        j, c = divmod(k, 128)
        i_k, u_k = int(ii[k]), int(uu[k])
        P_np[i_k, j, c] = sigma[i_k, u_k]
        Qn_np[c, j, u_k] = wpe[i_k, u_k]
        Qd_np[c, j, u_k] = wp[i_k, u_k]
        msig_np[c, j] = msig_d[i_k, u_k]

    p = {}
    p["_J"] = J  # not a tensor; popped before upload
    p["P"] = P_np
    p["Qn"] = Qn_np
    p["Qd"] = Qd_np
    p["msig"] = msig_np
    p["msig_rows"] = h(msig_np.T.reshape(1, J, 128))  # [1, J, 128] for rank-1 bias mms
    p["ones_row"] = h(np.ones((1, 128)))
    p["diagcmt"] = h(np.diag(cm_t))
    p["ident16"] = h(np.eye(128))
    p["glvl_b"] = f(np.broadcast_to((gl * vleak)[None, :], (B, U)))
    p["cmgl_eps_b"] = f(np.broadcast_to((cm_t + gl + EPS)[None, :], (B, U)))
    # sensory slabs in [b, u, i'] layout, broadcast over b
    p["ssig_s"] = f(np.broadcast_to(s_sigma.T[None], (B, U, I)))
    p["smsig_s"] = f(np.broadcast_to((-(s_mu * s_sigma)).T[None], (B, U, I)))
    p["swpe_s"] = f(np.broadcast_to(swpe.T[None], (B, U, I)))
    p["swp_s"] = f(np.broadcast_to(swp.T[None], (B, U, I)))
    # layernorm / input affine folded: inp = xn*effg + effb per feature
    effg = np.asarray(inputs["ln_g"], np.float64) * np.asarray(inputs["in_w"], np.float64)
    effb = (np.asarray(inputs["ln_b"], np.float64) * np.asarray(inputs["in_w"], np.float64)
            + np.asarray(inputs["in_b"], np.float64))
    p["effg_rep"] = f(np.broadcast_to(effg[None, None, :], (B, T, I)))
    p["effb_rep"] = f(np.broadcast_to(effb[None, None, :], (B, T, I)))
    p["outw"] = f(np.asarray(inputs["out_w"]).reshape(M, 1))
    p["outb"] = f(np.asarray(inputs["out_b"]).reshape(M, 1))
    p["aw1"] = f(inputs["aw1"])                       # [64,32]
    p["ab1"] = f(np.asarray(inputs["ab1"]).reshape(H1, 1))
    p["aw2"] = f(inputs["aw2"])                       # [32,1]
    p["cw1"] = f(inputs["cw1"])                       # [64,128]
    p["cb1"] = f(np.asarray(inputs["cb1"]).reshape(H2, 1))
    p["cw2"] = f(inputs["cw2"])                       # [128,100]
    p["cb2"] = f(np.asarray(inputs["cb2"]).reshape(C, 1))
    p["ident"] = f(np.eye(128))
    p["ones_m"] = f(np.ones((1, M)))
    return p


def _declare_inputs(nc, p):
    d = {}
    for name, arr in p.items():
        if name.startswith("_"):
            continue
        dt = F16 if arr.dtype == np.float16 else F32
        d[name] = nc.dram_tensor(name, list(arr.shape), dt, kind="ExternalInput").ap()
    d["x"] = nc.dram_tensor("x", [B, T, I], F32, kind="ExternalInput").ap()
    return d


def _build(nc, tc, d, J, n_steps=T, hw_loop=True, dbg=False):
    out_d = nc.dram_tensor("out", [B, C], F32, kind="ExternalOutput").ap()
    if dbg:
        dbg_v = nc.dram_tensor("dbg_v", [U, B], F16, kind="ExternalOutput").ap()
        dbg_wnT = nc.dram_tensor("dbg_wnT", [U, B], F16, kind="ExternalOutput").ap()
        dbg_wdT = nc.dram_tensor("dbg_wdT", [U, B], F16, kind="ExternalOutput").ap()
        dbg_outs = nc.dram_tensor("dbg_outs", [M, B, T], F32, kind="ExternalOutput").ap()
        dbg_inp = nc.dram_tensor("dbg_inp", [B, T + 1, I], F32, kind="ExternalOutput").ap()

    # chunking of the J k-chunks for args/sigmoid processing: small chunks at
    # the unfold boundaries shorten the serial gather->add->sigmoid->reduce
    # chain that restarts each unfold; 6-wide chunks in the middle.
    CH = 6
    AHEAD = 3  # gather-ahead depth (PSUM-buffer limited)
    if J > 12:
        rem = J - 5
        m, r = divmod(rem, 6)
        sizes = [3] + [6] * m + ([r] if r else []) + [2]
    else:
        sizes = [min(CH, J - j0) for j0 in range(0, J, CH)]
    chunks = []
    j0 = 0
    for n in sizes:
        chunks.append((j0, n))
        j0 += n
    assert j0 == J, (sizes, J)

    cpool = tc.alloc_tile_pool(name="consts", bufs=1)
    P_sb = cpool.tile([U, J, 128], F16)
    Qn_sb = cpool.tile([128, J, U], F16)
    Qd_sb = cpool.tile([128, J, U], F16)
    msig_sb = cpool.tile([128, J], F32)
    msig_rows_sb = cpool.tile([1, J, 128], F16)
    ones_row_sb = cpool.tile([1, 128], F16)
    diagcmt_sb = cpool.tile([128, 128], F16)
    ident16_sb = cpool.tile([128, 128], F16)
    glvl_b = cpool.tile([B, U], F32)
    cmgl_eps_b = cpool.tile([B, U], F32)
    ssig_s = cpool.tile([B, U, I], F32)
    smsig_s = cpool.tile([B, U, I], F32)
    swpe_s = cpool.tile([B, U, I], F32)
    swp_s = cpool.tile([B, U, I], F32)
    outw_sb = cpool.tile([M, 1], F32)
    outb_sb = cpool.tile([M, 1], F32)
    ident_sb = cpool.tile([128, 128], F32)
    for t_sb, name in [(P_sb, "P"), (Qn_sb, "Qn"), (Qd_sb, "Qd"), (msig_sb, "msig"),
                       (msig_rows_sb, "msig_rows"), (ones_row_sb, "ones_row"),
                       (diagcmt_sb, "diagcmt"), (ident16_sb, "ident16"),
                       (glvl_b, "glvl_b"), (cmgl_eps_b, "cmgl_eps_b"),
                       (ssig_s, "ssig_s"), (smsig_s, "smsig_s"), (swpe_s, "swpe_s"),
                       (swp_s, "swp_s"), (outw_sb, "outw"), (outb_sb, "outb"),
                       (ident_sb, "ident")]:
        nc.sync.dma_start(out=t_sb[:], in_=d[name])

    # ---------------- LN prologue -> inp slab [B, T+1, I] (last step zero) ----
    inp_slab = cpool.tile([B, T + 1, I], F32)
    nc.vector.memset(inp_slab[:], 0.0)
    x_sb = cpool.tile([B, T, I], F32)
    nc.sync.dma_start(out=x_sb[:], in_=d["x"])
    effg_sb = cpool.tile([B, T, I], F32)
    effb_sb = cpool.tile([B, T, I], F32)
    nc.sync.dma_start(out=effg_sb[:], in_=d["effg_rep"])
    nc.sync.dma_start(out=effb_sb[:], in_=d["effb_rep"])

    lnp = tc.alloc_tile_pool(name="ln", bufs=1)
    mean = lnp.tile([B, T, 1], F32)
    nc.vector.reduce_sum(mean[:, :, 0], x_sb[:], mybir.AxisListType.X)
    nc.vector.tensor_scalar_mul(mean[:], mean[:], 1.0 / I)
    xc = lnp.tile([B, T, I], F32)
    nc.vector.tensor_sub(xc[:], x_sb[:], mean[:].to_broadcast((B, T, I)))
    sq = lnp.tile([B, T, I], F32)
    nc.vector.tensor_mul(sq[:], xc[:], xc[:])
    ms = lnp.tile([B, T, 1], F32)
    nc.vector.reduce_sum(ms[:, :, 0], sq[:], mybir.AxisListType.X)
    sd = lnp.tile([B, T, 1], F32)
    ln_eps = lnp.tile([B, 1], F32)
    nc.vector.memset(ln_eps[:], 1e-5)
    nc.scalar.activation(sd[:], ms[:], AF.Sqrt, bias=ln_eps[:], scale=1.0 / I)
    rstd = lnp.tile([B, T, 1], F32)
    nc.vector.reciprocal(rstd[:], sd[:])
    xn = lnp.tile([B, T, I], F32)
    nc.vector.tensor_mul(xn[:], xc[:], rstd[:].to_broadcast((B, T, I)))
    nc.vector.tensor_mul(xn[:], xn[:], effg_sb[:])
    nc.vector.tensor_add(inp_slab[:, 0:T, :], xn[:], effb_sb[:])
    lnp.release()

    # ---------------- scan state ----------------
    v_T = cpool.tile([U, B], F16)
    nc.vector.memset(v_T[:], 0.0)
    outs_T = cpool.tile([M, B, T], F32)
    if dbg:
        nc.vector.memset(outs_T[:], 0.0)
    wnum_tot = cpool.tile([B, U], F32)   # [b,u] sens+leak totals
    wden_tot = cpool.tile([B, U], F32)
    wnumT = cpool.tile([U, B], F16)      # transposed for PE injection
    wdenT = cpool.tile([U, B], F16)

    spool = tc.alloc_tile_pool(name="sens", bufs=2)
    apool = tc.alloc_tile_pool(name="args", bufs=3)
    gpool = tc.alloc_tile_pool(name="g", bufs=3)
    pa_pool = tc.alloc_tile_pool(name="pa", bufs=AHEAD, space="PSUM")
    nd_pool = tc.alloc_tile_pool(name="pnd", bufs=2, space="PSUM")

    def sens_stages(t_idx):
        """Stage list computing wnum_tot/wden_tot ([b,u]) for step t_idx.

        Split into small pieces interleaved between unfolds so the big Pool
        ops never head-of-line block the unfold-chunk adds on the Pool queue.
        """
        inp_t = inp_slab[:, bass.ds(t_idx, 1), :]  # [B, 1, I] -> broadcast over u
        sarg = spool.tile([B, U, I], F32, name=f"sarg{t_idx if isinstance(t_idx, int) else 'r'}")
        ssg = spool.tile([B, U, I], F32, name="ssg_t")
        tmp = spool.tile([B, U, I], F32, name="tmp_t")
        tmp2 = spool.tile([B, U, I], F32, name="tmp2_t")
        wns = spool.tile([B, U, 1], F32, name="wns_t")
        wds = spool.tile([B, U, 1], F32, name="wds_t")
        return [
            lambda: nc.gpsimd.tensor_mul(sarg[:], inp_t.to_broadcast((B, U, I)), ssig_s[:]),
            lambda: (nc.gpsimd.tensor_add(sarg[:], sarg[:], smsig_s[:]),
                     nc.scalar.activation(ssg[:], sarg[:], AF.Sigmoid)),
            lambda: (nc.gpsimd.tensor_mul(tmp[:], ssg[:], swpe_s[:]),
                     nc.vector.reduce_sum(wns[:, :, 0], tmp[:], mybir.AxisListType.X)),
            lambda: (nc.gpsimd.tensor_mul(tmp2[:], ssg[:], swp_s[:]),
                     nc.vector.reduce_sum(wds[:, :, 0], tmp2[:], mybir.AxisListType.X)),
            lambda: (nc.gpsimd.tensor_add(wnum_tot[:], wns[:, :, 0], glvl_b[:]),
                     nc.gpsimd.tensor_add(wden_tot[:], wds[:, :, 0], cmgl_eps_b[:])),
        ]

    def sens_block(t_idx):
        for f in sens_stages(t_idx):
            f()

    def sens_transpose():
        # transpose to [u, b] fp16 for PE injection
        pt1 = nd_pool.tile([U, B], F32, tag="ps")
        nc.tensor.transpose(pt1[:], wnum_tot[:], ident_sb[:])
        nc.vector.tensor_copy(wnumT[:], pt1[:])
        pt2 = nd_pool.tile([U, B], F32, tag="ps")
        nc.tensor.transpose(pt2[:], wden_tot[:], ident_sb[:])
        nc.vector.tensor_copy(wdenT[:], pt2[:])

    sens_block(0)
    sens_transpose()

    PEBIAS = 3  # leading chunks whose msig lands via PE rank-1 matmuls
    # the final chunk also gets PE-side bias: its sigmoid is on the unfold
    # drain chain, so keeping DVE out of it shortens the restart
    pebias_set = set(range(PEBIAS)) | {len(chunks) - 1}

    def gathers(pa, j0, n, with_bias=False):
        for jl in range(n):
            j = j0 + jl
            if with_bias:
                # args = P_j^T v + msig_j assembled fully in PSUM: ACT can
                # read it directly, keeping DVE off the unfold-restart chain
                nc.tensor.matmul(pa[:, jl, :], lhsT=P_sb[:, j, :], rhs=v_T[:],
                                 start=True, stop=False)
                nc.tensor.matmul(pa[:, jl, :], lhsT=msig_rows_sb[:, j, :],
                                 rhs=ones_row_sb[:], start=False, stop=True)
            else:
                nc.tensor.matmul(pa[:, jl, :], lhsT=P_sb[:, j, :], rhs=v_T[:],
                                 start=True, stop=True)

    def unfold_body(transpose_mid=False):
        nd = nd_pool.tile([U, 2, B], F32, tag="ps")  # [:,0,:]=num, [:,1,:]=den
        pas = {}
        # first gathers go ahead of the injects on the PE queue: the sigmoid
        # pipeline restart is the critical chain after the v update
        pas[0] = pa_pool.tile([128, CH, B], F32, tag="ps", name="pa0")
        gathers(pas[0], chunks[0][0], chunks[0][1], with_bias=0 in pebias_set)
        # num+den share one PSUM bank => a single accumulation group: the
        # bank-wide start from the diag matmul covers den (its first write
        # lands on pending-zero bytes), and only the last Qd carries stop.
        nc.tensor.matmul(nd[:, 0, :], lhsT=diagcmt_sb[:], rhs=v_T[:],
                         start=True, stop=False)
        nc.tensor.matmul(nd[:, 0, :], lhsT=ident16_sb[:], rhs=wnumT[:],
                         start=False, stop=False)
        nc.tensor.matmul(nd[:, 1, :], lhsT=ident16_sb[:], rhs=wdenT[:],
                         start=False, stop=False)
        for ci in range(1, min(AHEAD, len(chunks))):
            pas[ci] = pa_pool.tile([128, CH, B], F32, tag="ps", name=f"pa{ci}")
            gathers(pas[ci], chunks[ci][0], chunks[ci][1], with_bias=ci in pebias_set)
        def msig_bc(j0, n0, n1):
            return (msig_sb[:, j0 + n0:j0 + n1]
                    .rearrange("p (j o) -> p j o", o=1).to_broadcast((128, n1 - n0, B)))
        for ci, (j0, n) in enumerate(chunks):
            pa = pas.pop(ci)
            if ci in pebias_set:
                g_src = pa[:, 0:n, :]  # bias already injected on PE
            else:
                # GPSIMD cannot access PSUM (neuronxcc BIR verifier) -> DVE
                args_sb = apool.tile([128, CH, B], F16)
                nc.vector.tensor_add(args_sb[:, 0:n, :], pa[:, 0:n, :], msig_bc(j0, 0, n))
                g_src = args_sb[:, 0:n, :]
            g_sb = gpool.tile([128, CH, B], F16)
            nc.scalar.activation(g_sb[:, 0:n, :], g_src, AF.Sigmoid)
            if transpose_mid and ci == 1:
                # next step's sens transposes: after this unfold's injects
                # (last wnumT readers), away from the fill/drain chains
                sens_transpose()
            if ci + AHEAD < len(chunks):
                pas[ci + AHEAD] = pa_pool.tile([128, CH, B], F32, tag="ps", name=f"pa{ci + AHEAD}")
                gathers(pas[ci + AHEAD], chunks[ci + AHEAD][0], chunks[ci + AHEAD][1],
                        with_bias=ci + AHEAD in pebias_set)
            if ci == len(chunks) - 1:
                # final chunk: all den reduces first so the reciprocal can
                # start while the num reduces still stream; group stop moves
                # to the last num matmul
                for jl in range(n):
                    nc.tensor.matmul(nd[:, 1, :], lhsT=Qd_sb[:, j0 + jl, :],
                                     rhs=g_sb[:, jl, :], start=False, stop=False)
                for jl in range(n):
                    nc.tensor.matmul(nd[:, 0, :], lhsT=Qn_sb[:, j0 + jl, :],
                                     rhs=g_sb[:, jl, :], start=False,
                                     stop=jl == n - 1)
            else:
                for jl in range(n):
                    j = j0 + jl
                    nc.tensor.matmul(nd[:, 0, :], lhsT=Qn_sb[:, j, :], rhs=g_sb[:, jl, :],
                                     start=False, stop=False)
                    nc.tensor.matmul(nd[:, 1, :], lhsT=Qd_sb[:, j, :], rhs=g_sb[:, jl, :],
                                     start=False, stop=False)
        # v = num/den: two instructions — DVE may read only one PSUM operand
        # per instruction on HW (NCC_IBVF027)
        rec = apool.tile([U, B], F32, name="rec")
        nc.vector.reciprocal(rec[:], nd[:, 1, :])
        nc.vector.tensor_mul(v_T[:], nd[:, 0, :], rec[:])

    def step_body(t):
        for _k in range(UNFOLDS - 1):
            unfold_body()
        # sensory precompute for t+1 overlaps the unfolds; its transposes are
        # emitted inside the last unfold, right after the final wnumT readers
        sens_block(t + 1)
        unfold_body(transpose_mid=True)
        # outs_T[:, :, t] = v_T[:64] * out_w + out_b
        nc.vector.tensor_scalar(
            out=outs_T[:, :, bass.ds(t, 1)],
            in0=v_T[0:M, :].rearrange("p (b o) -> p b o", o=1),
            scalar1=outw_sb[:], scalar2=outb_sb[:], op0=OP.mult, op1=OP.add)

    if hw_loop:
        with tc.For_i(0, n_steps, 1) as t:
            step_body(t)
    else:
        for t in range(n_steps):
            step_body(t)

    if dbg:
        nc.sync.dma_start(out=dbg_v, in_=v_T[:])
        nc.sync.dma_start(out=dbg_wnT, in_=wnumT[:])
        nc.sync.dma_start(out=dbg_wdT, in_=wdenT[:])
        nc.sync.dma_start(out=dbg_outs, in_=outs_T[:])
        nc.sync.dma_start(out=dbg_inp, in_=inp_slab[:])

    for pool in (nd_pool, pa_pool, gpool, apool, spool):
        pool.release()

    # ---------------- attention pooling + classifier ----------------
    aw1_sb = cpool.tile([M, H1], F32)
    ab1_sb = cpool.tile([H1, 1], F32)
    aw2_sb = cpool.tile([H1, 1], F32)
    cw1_sb = cpool.tile([M, H2], F32)
    cb1_sb = cpool.tile([H2, 1], F32)
    cw2_sb = cpool.tile([H2, C], F32)
    cb2_sb = cpool.tile([C, 1], F32)
    ones_sb = cpool.tile([1, M], F32)
    for t_sb, name in [(aw1_sb, "aw1"), (ab1_sb, "ab1"), (aw2_sb, "aw2"),
                       (cw1_sb, "cw1"), (cb1_sb, "cb1"), (cw2_sb, "cw2"),
                       (cb2_sb, "cb2"), (ones_sb, "ones_m")]:
        nc.sync.dma_start(out=t_sb[:], in_=d[name])

    epool = tc.alloc_tile_pool(name="ep", bufs=2)
    e1pool = tc.alloc_tile_pool(name="e1", bufs=1)
    ps_h = tc.alloc_tile_pool(name="psh", bufs=2, space="PSUM")
    ps_s = tc.alloc_tile_pool(name="pss", bufs=2, space="PSUM")

    outs_flat = outs_T[:].rearrange("p b t -> p (b t)")
    dpool = tc.alloc_tile_pool(name="dscr", bufs=1, space="DRAM")
    scr1 = dpool.tile([1, B * T], F32)
    NC1 = 512
    for c in range(B * T // NC1):
        hp = ps_h.tile([H1, NC1], F32, tag="ps")
        nc.tensor.matmul(hp[:], lhsT=aw1_sb[:], rhs=outs_flat[:, c * NC1:(c + 1) * NC1],
                         start=True, stop=True)
        hs = epool.tile([H1, NC1], F32)
        nc.scalar.activation(hs[:], hp[:], AF.Relu, bias=ab1_sb[:])
        sp = ps_s.tile([1, NC1], F32, tag="ps")
        nc.tensor.matmul(sp[:], lhsT=aw2_sb[:], rhs=hs[:], start=True, stop=True)
        sc = epool.tile([1, NC1], F32)
        nc.vector.tensor_copy(sc[:], sp[:])
        nc.sync.dma_start(out=scr1[:, c * NC1:(c + 1) * NC1], in_=sc[:])

    # softmax over t, per b: scores land in DRAM as [1, (b t)]; reload as [b, t]
    scores_bt = e1pool.tile([B, T], F32)
    nc.sync.dma_start(out=scores_bt[:],
                      in_=scr1[:].rearrange("o (b t) -> (o b) t", b=B))
    mx = e1pool.tile([B, 1], F32)
    nc.vector.reduce_max(mx[:], scores_bt[:], mybir.AxisListType.X)
    es = e1pool.tile([B, T], F32)
    nc.vector.tensor_scalar(out=es[:], in0=scores_bt[:], scalar1=mx[:],
                            scalar2=None, op0=OP.subtract)
    nc.scalar.activation(es[:], es[:], AF.Exp)
    ssum = e1pool.tile([B, 1], F32)
    nc.vector.reduce_sum(ssum[:], es[:], mybir.AxisListType.X)
    rs = e1pool.tile([B, 1], F32)
    nc.vector.reciprocal(rs[:], ssum[:])
    attn_bt = e1pool.tile([B, T], F32)
    nc.vector.tensor_scalar(out=attn_bt[:], in0=es[:], scalar1=rs[:],
                            scalar2=None, op0=OP.mult)
    scr2 = dpool.tile([B, T], F32)
    nc.sync.dma_start(out=scr2[:], in_=attn_bt[:])
    attn_flat = e1pool.tile([1, B * T], F32)
    nc.sync.dma_start(out=attn_flat[:], in_=scr2[:].rearrange("b t -> (b t)").rearrange("(o n) -> o n", o=1))

    # ctx_T[m, b] = sum_t outs_T[m,b,t] * attn[b,t]
    ctx_T = e1pool.tile([M, B], F32)
    NB = 4
    for c in range(B // NB):
        ap_ps = ps_h.tile([M, NB * T], F32, tag="ps")
        nc.tensor.matmul(ap_ps[:], lhsT=ones_sb[:],
                         rhs=attn_flat[:, c * NB * T:(c + 1) * NB * T],
                         start=True, stop=True)
        wo = epool.tile([M, NB, T], F32)
        nc.vector.tensor_mul(wo[:], outs_T[:, c * NB:(c + 1) * NB, :],
                             ap_ps[:].rearrange("p (b t) -> p b t", t=T))
        nc.vector.reduce_sum(ctx_T[:, c * NB:(c + 1) * NB], wo[:], mybir.AxisListType.X)

    # classifier
    h2p = ps_h.tile([H2, B], F32, tag="ps")
    nc.tensor.matmul(h2p[:], lhsT=cw1_sb[:], rhs=ctx_T[:], start=True, stop=True)
    h2 = e1pool.tile([H2, B], F32)
    nc.scalar.activation(h2[:], h2p[:], AF.Relu, bias=cb1_sb[:])
    zp = ps_h.tile([C, B], F32, tag="ps")
    nc.tensor.matmul(zp[:], lhsT=cw2_sb[:], rhs=h2[:], start=True, stop=True)
    zT = e1pool.tile([C, B], F32)
    nc.scalar.activation(zT[:], zp[:], AF.Identity, bias=cb2_sb[:])
    tp = ps_h.tile([B, C], F32, tag="ps")
    nc.tensor.matmul(tp[:], lhsT=zT[:], rhs=ident_sb[0:C, 0:C], is_transpose=True,
                     start=True, stop=True)
    zf = e1pool.tile([B, C], F32)
    nc.vector.tensor_copy(zf[:], tp[:])
    nc.sync.dma_start(out=out_d, in_=zf[:])

    for pool in (dpool, ps_s, ps_h, e1pool, epool, cpool):
        pool.release()


_CACHE = {}


def _get_compiled(p, n_steps=T, hw_loop=True, dbg=False):
    key = ("nc", n_steps, hw_loop, dbg)
    if key in _CACHE:
        return _CACHE[key]
    nc = bacc.Bacc("TRN2", target_bir_lowering=False, debug=False,
                   enable_asserts=False)
    d = _declare_inputs(nc, p)
    with tile.TileContext(nc) as tc:
        _build(nc, tc, d, p["_J"], n_steps=n_steps, hw_loop=hw_loop, dbg=dbg)
    nc.compile()
    _CACHE[key] = nc
    return nc


def kernel(**inputs):
    global LAST_RESULTS
    p = _build_params(inputs)
    nc = _get_compiled(p)
    x = np.ascontiguousarray(np.asarray(inputs["x"], np.float32))
    pt = {k: v for k, v in p.items() if not k.startswith("_")}
    in_maps = []
    for ci in range(N_CORES):
        m = dict(pt)
        m["x"] = np.ascontiguousarray(x[ci * B:(ci + 1) * B])
        in_maps.append(m)
    res = bass_utils.run_bass_kernel_spmd(
        nc, in_maps, core_ids=list(range(N_CORES)), trace=TRACE)
    LAST_RESULTS = res
    out = np.concatenate([res.results[ci]["out"] for ci in range(N_CORES)], axis=0)
    return out.astype(np.float32)
